# revision 1
# baseline (speedup 1.0000x reference)
"""Trainium2 Bass kernel for nn_CACProjector (logits = x @ W^T, CAC distances).

Data-parallel over batch B across 8 NeuronCores; each core handles a
(768, 2048) column-slice xT of x^T plus a replicated W^T (768, 1024), all
shipped bf16:

  logits[b, c] = sum_d xT[d, b] * wT[d, c]      (PE, fp32 accumulate in PSUM)
  sq_norm[b]   = alpha^2 + sum_c logits[b, c]^2 (DVE tensor_tensor_reduce,
                                                 ONE pass: square + row-reduce
                                                 + alpha^2 init)
  dist[b, c]   = sqrt(sq_norm[b] - 2*alpha*logits[b, c])   (ACT Sqrt)

Engine split per 128-row b-tile: PE 12 matmuls (~2.6 us, pacing); the
PSUM->bf16 logits copy alternates between ACT and DVE (neither in-order queue
saturates); DVE runs one fused square+row-accumulate (scalar_tensor_tensor
accum_out) plus a [P,1] alpha^2 add; ACT runs the Sqrt. Tiles 13-15 are
"drain" tiles: Square-with-accumulate and Sqrt read PSUM directly on ACT so no
copy sits on the final dependency chain, and the last tile's Sqrt/store is
split in halves. (GPSIMD cannot access PSUM on TRN2, and tensor_tensor_reduce
dies at execution on this stack -- both found the hard way.)

Logits and dist are written into ONE [128, 2048] SBUF tile (lg || dist) and
shipped with a single DMA per tile into a fused [2048, 2048] DRAM tensor
(4 KB contiguous per row); the host splits the halves.

Input DMAs are issued in PE consumption order: (w_k, x_k quarter-0) pairs feed
a k-major opening over b-tiles 0-3 (each arriving pair unlocks 8 matmuls), and
x quarters 1-3 follow as single rearranged transfers. Dummy matmuls on a
zeroed tile bridge the NEFF preamble so the PE clock is ramped when real data
lands; a throwaway Sqrt preloads the ACT table off the first epilogue's path.

The final b-tile's epilogue is split into 512-column halves chained through
tensor_tensor_reduce's initial-value operand so the drain after the last
matmul is ~3 us instead of ~6; its dist store is kicked from the ACT queue
(ACT is an HWDGE engine) right behind the Sqrt.

d2 = ||l||^2 - 2a*l_j + a^2 >= (l_j - a)^2 >= 0 and with this data d2 ~ 1100,
so the reference's maximum(d2, 0) clamp is a no-op.
"""

import sys

sys.path.insert(0, "/opt/trn_rl_repo")

from contextlib import ExitStack

import ml_dtypes
import numpy as np

import concourse.tile as tile
from concourse import bacc, mybir
from concourse.bass_utils import run_bass_kernel_spmd

N_CORES = 8
B, D, C = 16384, 768, 1024
BS = B // N_CORES          # 2048 rows of B per core
P = 128                    # partition dim
KT = D // P                # 6 contraction chunks
NBT = BS // P              # 16 output row-tiles per core
Q = 512                    # b-quarter width (4 tiles)
ALPHA = 10.0

F32 = mybir.dt.float32
BF16 = mybir.dt.bfloat16


def build():
    nc = bacc.Bacc("TRN2", target_bir_lowering=False, debug=False)
    xT = nc.dram_tensor("xT", [D, BS], BF16, kind="ExternalInput").ap()
    wT = nc.dram_tensor("wT", [D, C], BF16, kind="ExternalInput").ap()
    out = nc.dram_tensor("out", [BS, 2 * C], BF16, kind="ExternalOutput").ap()

    mult, add = mybir.AluOpType.mult, mybir.AluOpType.add
    SQRT = mybir.ActivationFunctionType.Sqrt

    with tile.TileContext(nc) as tc, ExitStack() as ctx:
        xpool = ctx.enter_context(tc.tile_pool(name="xT", bufs=1))
        wpool = ctx.enter_context(tc.tile_pool(name="wT", bufs=1))
        psum = ctx.enter_context(tc.tile_pool(name="psum", bufs=4, space="PSUM"))
        lgpool = ctx.enter_context(tc.tile_pool(name="lgdt", bufs=8))
        spool = ctx.enter_context(tc.tile_pool(name="sq", bufs=4))
        npool = ctx.enter_context(tc.tile_pool(name="norms", bufs=10))

        # PE warm-up operand: memset on GPSIMD, whose framework preamble ends
        # earliest, so the dummy matmuls can start ~6.4 us and have the PE
        # clock ramped before the first input chunk lands (~9 us).
        warm = xpool.tile([P, Q], BF16, tag="warm")
        nc.gpsimd.memset(warm[:], 0)

        # ---- input DMAs, issued in PE consumption order ----
        # w kicks ride the sync queue, x kicks the scalar queue (also HWDGE):
        # two dispatchers halve the serial ~0.65us/kick cost at the front.
        # w0 is two separate TILES: tile-granular dep tracking would otherwise
        # gate the first lo-half matmul on the hi half's arrival.
        w0lo = wpool.tile([P, 512], BF16, tag="w0lo")
        w0hi = wpool.tile([P, 512], BF16, tag="w0hi")
        wt = [
            wpool.tile([P, C], BF16, tag=f"w{k}", name=f"w{k}")
            for k in range(1, KT)
        ]
        xq0 = [
            xpool.tile([P, Q], BF16, tag=f"xq0_{k}", name=f"xq0_{k}")
            for k in range(KT)
        ]
        # Quarter 1 ships as per-k pieces so the k-major group over tiles
        # 4-7 can start on partial data; quarters 2-3 land far ahead of use
        # and ride single rearranged transfers. xq1 kicks interleave behind
        # the tail of the q0 pieces so group B's early rounds aren't starved.
        xq1 = [
            xpool.tile([P, Q], BF16, tag=f"xq1_{k}", name=f"xq1_{k}")
            for k in range(KT)
        ]
        nc.sync.dma_start(w0lo[:], wT[0:P, 0:512])
        nc.scalar.dma_start(xq0[0][:], xT[0:P, 0:Q])
        nc.sync.dma_start(w0hi[:], wT[0:P, 512:1024])
        for k in range(1, KT):
            nc.sync.dma_start(wt[k - 1][:], wT[k * P : (k + 1) * P, :])
            nc.scalar.dma_start(xq0[k][:], xT[k * P : (k + 1) * P, 0:Q])
        for k in range(KT):
            nc.scalar.dma_start(xq1[k][:], xT[k * P : (k + 1) * P, Q : 2 * Q])
        xq = {0: None, 1: None}
        for q in range(2, 4):
            t_ = xpool.tile([P, KT * Q], BF16, tag=f"xq{q}")
            nc.scalar.dma_start(
                t_[:].rearrange("p (k b) -> p k b", k=KT),
                xT[:, q * Q : (q + 1) * Q].rearrange("(k p) b -> p k b", p=P),
            )
            xq[q] = t_

        def w_slice(k, half):
            if k == 0:
                return (w0lo if half == 0 else w0hi)[:]
            return wt[k - 1][:, half * 512 : (half + 1) * 512]

        def x_slice(k, bt):
            q, boff = divmod(bt * P, Q)
            if q == 0:
                return xq0[k][:, boff : boff + P]
            if q == 1:
                return xq1[k][:, boff : boff + P]
            return xq[q][:, k * Q + boff : k * Q + boff + P]

        # ACT Sqrt table preload, off the first epilogue's critical path.
        scrap = npool.tile([P, 1], F32, tag="scrap")
        nc.scalar.activation(scrap[:], warm[:, 0:1], SQRT)

        # ---- PE clock ramp: dummies bridge preamble-end -> first data ----
        pss = [psum.tile([P, C], F32, tag="ps", name=f"ps{i}") for i in range(4)]
        # Enough dummies that the first real matmul fires ~11.5us with a hot
        # clock AND a DMA delivery cushion: starting real work earlier just
        # trades dummy time for a clock-resetting data stall.
        for _ in range(11):
            nc.tensor.matmul(
                pss[3][:, 512:1024],
                warm[:, 0:P],
                warm[:],
                start=True,
                stop=True,
                skip_group_check=True,
            )

        def mm(bt, ps, k):
            lhs = x_slice(k, bt)
            nc.tensor.matmul(
                ps[:, 0:512], lhs, w_slice(k, 0), start=(k == 0), stop=(k == KT - 1)
            )
            nc.tensor.matmul(
                ps[:, 512:1024], lhs, w_slice(k, 1),
                start=(k == 0), stop=(k == KT - 1),
            )

        SQUARE = mybir.ActivationFunctionType.Square

        def epilogue(bt, ps):
            # Steady state (t0-t12): psum->bf16 logits copy alternating
            # ACT/DVE, one fused DVE square+row-accumulate from the bf16
            # logits, [P,1] alpha^2 add, ACT Sqrt, one fused logits||dist
            # store. Everything stays ~1-2.3us/engine per 2.56us tile.
            lg = lgpool.tile([P, 2 * C], BF16, tag="lgdt")
            sq = spool.tile([P, C], BF16, tag="sq")
            sn = npool.tile([P, 1], F32, tag="sn")
            snb = npool.tile([P, 1], F32, tag="snb")
            if bt % 2 == 0:
                nc.scalar.copy(lg[:, 0:1024], ps[:])
            else:
                nc.vector.tensor_copy(lg[:, 0:1024], ps[:])
            nc.vector.scalar_tensor_tensor(
                sq[:], lg[:, 0:1024], 1.0, lg[:, 0:1024],
                mult, mult, accum_out=sn[:],
            )
            nc.vector.tensor_scalar_add(snb[:], sn[:], ALPHA * ALPHA)
            nc.scalar.activation(
                lg[:, 1024:2048], lg[:, 0:1024], SQRT,
                bias=snb[:], scale=-2.0 * ALPHA,
            )
            nc.sync.dma_start(out[bt * P : (bt + 1) * P, :], lg[:])

        def drain_epilogue(bt, ps):
            # Drain tiles (t13-t15): everything dist-critical reads PSUM
            # directly -- ACT Square-with-accumulate then Sqrt-from-psum --
            # so no copy sits on the critical chain; the bf16 logits cast
            # rides the DVE into its OWN tile (shared tiles would add a
            # false tile-granular WAW dep) and ships separately.
            lgt = lgpool.tile([P, C], BF16, tag="lgt")
            dt_ = lgpool.tile([P, C], BF16, tag="dt")
            sq = spool.tile([P, C], BF16, tag="sq")
            snb = npool.tile([P, 1], F32, tag="snb")
            rows = slice(bt * P, (bt + 1) * P)
            if bt < NBT - 1:
                sn = npool.tile([P, 1], F32, tag="sn")
                nc.scalar.activation(sq[:], ps[:], SQUARE, accum_out=sn[:])
                nc.vector.tensor_scalar_add(snb[:], sn[:], ALPHA * ALPHA)
                nc.vector.tensor_copy(lgt[:], ps[:])
                nc.sync.dma_start(out[rows, 0:1024], lgt[:])
                nc.scalar.activation(
                    dt_[:], ps[:], SQRT, bias=snb[:], scale=-2.0 * ALPHA
                )
                nc.sync.dma_start(out[rows, 1024:2048], dt_[:])
                return
            # Final tile: one full-width Square+accumulate at the last
            # matmul's retirement; the bf16 logits cast rides the DVE in
            # parallel and ships first; dist halves kick as each Sqrt half
            # retires.
            sn = npool.tile([P, 1], F32, tag="sn")
            nc.vector.tensor_copy(lgt[:], ps[:])
            nc.sync.dma_start(out[rows, 0:1024], lgt[:])
            nc.scalar.activation(sq[:], ps[:], SQUARE, accum_out=sn[:])
            nc.vector.tensor_scalar_add(snb[:], sn[:], ALPHA * ALPHA)
            nc.scalar.activation(
                dt_[:, 0:512], ps[:, 0:512], SQRT, bias=snb[:], scale=-2.0 * ALPHA
            )
            nc.sync.dma_start(out[rows, 1024:1536], dt_[:, 0:512])
            nc.scalar.activation(
                dt_[:, 512:1024], ps[:, 512:1024], SQRT,
                bias=snb[:], scale=-2.0 * ALPHA,
            )
            nc.sync.dma_start(out[rows, 1536:2048], dt_[:, 512:1024])

        # b-tiles 0-3 run k-major so each arriving (w_k, x_k q0) pair unlocks
        # 8 matmuls; later tiles run tile-major once DMA is ahead of the PE.
        for k in range(KT):
            for t in range(4):
                mm(t, pss[t], k)
        for t in range(4):
            epilogue(t, pss[t])

        # tiles 4-7 also run k-major (on the per-k q1 pieces) so the PE can
        # start as soon as each chunk lands instead of waiting for all of q1.
        pss2 = [psum.tile([P, C], F32, tag="ps", name=f"psB{i}") for i in range(4)]
        for k in range(KT):
            for i in range(4):
                mm(4 + i, pss2[i], k)
        for i in range(4):
            epilogue(4 + i, pss2[i])

        for bt in range(8, NBT):
            ps = psum.tile([P, C], F32, tag="ps")
            for k in range(KT):
                mm(bt, ps, k)
            if bt >= 13:
                drain_epilogue(bt, ps)
            else:
                epilogue(bt, ps)

    nc.compile()
    return nc


_NC = {}


def kernel(x, W, trace=False, _result_box=None):
    if "nc" not in _NC:
        _NC["nc"] = build()
    nc = _NC["nc"]

    x = np.ascontiguousarray(np.asarray(x, dtype=np.float32))
    W = np.ascontiguousarray(np.asarray(W, dtype=np.float32))
    prep = lambda a: np.asarray(a, dtype=ml_dtypes.bfloat16)
    wT = prep(np.ascontiguousarray(W.T))
    in_maps = [
        {
            "xT": prep(np.ascontiguousarray(x[i * BS : (i + 1) * BS, :].T)),
            "wT": wT,
        }
        for i in range(N_CORES)
    ]

    # The first execution of a freshly loaded NEFF has been seen to flake
    # (transient NRT_EXEC_UNIT_UNRECOVERABLE / corrupt output on this
    # fabric); do a throwaway warm-up exec with one retry, then the real run.
    try:
        run_bass_kernel_spmd(nc, in_maps, list(range(N_CORES)))
    except Exception:
        try:
            run_bass_kernel_spmd(nc, in_maps, list(range(N_CORES)))
        except Exception:
            pass

    res = run_bass_kernel_spmd(nc, in_maps, list(range(N_CORES)), trace=trace)
    if _result_box is not None:
        _result_box.append(res)

    outs = [np.asarray(res.results[i]["out"], dtype=np.float32) for i in range(N_CORES)]
    logits = np.concatenate([o[:, :C] for o in outs], axis=0)
    dist = np.concatenate([o[:, C:] for o in outs], axis=0)
    return logits, dist



# revision 11
# speedup vs baseline: 1.0947x; 1.0947x over previous
"""Trainium2 Bass kernel for nn_CACProjector (logits = x @ W^T, CAC distances).

Data-parallel over batch B across 8 NeuronCores; each core handles a
(768, 2048) column-slice xT of x^T plus a replicated W^T (768, 1024), all
shipped bf16:

  logits[b, c] = sum_d xT[d, b] * wT[d, c]      (PE, fp32 accumulate in PSUM)
  sq_norm[b]   = alpha^2 + sum_c logits[b, c]^2 (DVE tensor_tensor_reduce,
                                                 ONE pass: square + row-reduce
                                                 + alpha^2 init)
  dist[b, c]   = sqrt(sq_norm[b] - 2*alpha*logits[b, c])   (ACT Sqrt)

Engine split per 128-row b-tile: PE 12 matmuls (~2.6 us, pacing); the
PSUM->bf16 logits copy alternates between ACT and DVE (neither in-order queue
saturates); DVE runs one fused square+row-accumulate (scalar_tensor_tensor
accum_out) plus a [P,1] alpha^2 add; ACT runs the Sqrt. Tiles 13-15 are
"drain" tiles: Square-with-accumulate and Sqrt read PSUM directly on ACT so no
copy sits on the final dependency chain, and the last tile's Sqrt/store is
split in halves. (GPSIMD cannot access PSUM on TRN2, and tensor_tensor_reduce
dies at execution on this stack -- both found the hard way.)

Logits and dist are written into ONE [128, 2048] SBUF tile (lg || dist) and
shipped with a single DMA per tile into a fused [2048, 2048] DRAM tensor
(4 KB contiguous per row); the host splits the halves.

Input DMAs are issued in PE consumption order: (w_k, x_k quarter-0) pairs feed
a k-major opening over b-tiles 0-3 (each arriving pair unlocks 8 matmuls), and
x quarters 1-3 follow as single rearranged transfers. Dummy matmuls on a
zeroed tile bridge the NEFF preamble so the PE clock is ramped when real data
lands; a throwaway Sqrt preloads the ACT table off the first epilogue's path.

The final b-tile's epilogue is split into 512-column halves chained through
tensor_tensor_reduce's initial-value operand so the drain after the last
matmul is ~3 us instead of ~6; its dist store is kicked from the ACT queue
(ACT is an HWDGE engine) right behind the Sqrt.

d2 = ||l||^2 - 2a*l_j + a^2 >= (l_j - a)^2 >= 0 and with this data d2 ~ 1100,
so the reference's maximum(d2, 0) clamp is a no-op.
"""

import sys

sys.path.insert(0, "/opt/trn_rl_repo")

from contextlib import ExitStack

import ml_dtypes
import numpy as np

import concourse.tile as tile
from concourse import bacc, mybir
from concourse.bass_utils import run_bass_kernel_spmd

N_CORES = 8
B, D, C = 16384, 768, 1024
BS = B // N_CORES          # 2048 rows of B per core
P = 128                    # partition dim
KT = D // P                # 6 contraction chunks
NBT = BS // P              # 16 output row-tiles per core
Q = 512                    # b-quarter width (4 tiles)
ALPHA = 10.0

F32 = mybir.dt.float32
BF16 = mybir.dt.bfloat16


def build():
    nc = bacc.Bacc("TRN2", target_bir_lowering=False, debug=False)
    xT = nc.dram_tensor("xT", [D, BS], BF16, kind="ExternalInput").ap()
    wT = nc.dram_tensor("wT", [D, C], BF16, kind="ExternalInput").ap()
    out = nc.dram_tensor("out", [BS, 2 * C], BF16, kind="ExternalOutput").ap()

    mult, add = mybir.AluOpType.mult, mybir.AluOpType.add
    SQRT = mybir.ActivationFunctionType.Sqrt

    with tile.TileContext(nc) as tc, ExitStack() as ctx:
        xpool = ctx.enter_context(tc.tile_pool(name="xT", bufs=1))
        wpool = ctx.enter_context(tc.tile_pool(name="wT", bufs=1))
        psum = ctx.enter_context(tc.tile_pool(name="psum", bufs=4, space="PSUM"))
        lgpool = ctx.enter_context(tc.tile_pool(name="lgdt", bufs=8))
        spool = ctx.enter_context(tc.tile_pool(name="sq", bufs=4))
        npool = ctx.enter_context(tc.tile_pool(name="norms", bufs=10))

        # PE warm-up operand: memset on GPSIMD, whose framework preamble ends
        # earliest, FIRST in the gpsimd queue so the dummy matmuls can start
        # ~7.5 us. (Tile refuses reads from never-written tiles, so the
        # memset cannot be skipped.)
        warm = xpool.tile([P, Q], BF16, tag="warm")
        nc.gpsimd.memset(warm[:], 0)

        # ---- input DMAs, issued in PE consumption order ----
        # w kicks ride the sync queue, x kicks the scalar queue (also HWDGE):
        # two dispatchers halve the serial ~0.65us/kick cost at the front.
        # w0 is two separate TILES: tile-granular dep tracking would otherwise
        # gate the first lo-half matmul on the hi half's arrival.
        w0lo = wpool.tile([P, 512], BF16, tag="w0lo")
        w0hi = wpool.tile([P, 512], BF16, tag="w0hi")
        wt = [
            wpool.tile([P, C], BF16, tag=f"w{k}", name=f"w{k}")
            for k in range(1, KT)
        ]
        xq0 = [
            xpool.tile([P, Q], BF16, tag=f"xq0_{k}", name=f"xq0_{k}")
            for k in range(KT)
        ]
        # Quarter 1 ships as per-k pieces so the k-major group over tiles
        # 4-7 can start on partial data; quarters 2-3 land far ahead of use
        # and ride single rearranged transfers. xq1 kicks interleave behind
        # the tail of the q0 pieces so group B's early rounds aren't starved.
        xq1 = [
            xpool.tile([P, Q], BF16, tag=f"xq1_{k}", name=f"xq1_{k}")
            for k in range(KT)
        ]
        nc.sync.dma_start(w0lo[:], wT[0:P, 0:512])
        nc.scalar.dma_start(xq0[0][:], xT[0:P, 0:Q])
        nc.sync.dma_start(w0hi[:], wT[0:P, 512:1024])
        for k in range(1, KT):
            nc.sync.dma_start(wt[k - 1][:], wT[k * P : (k + 1) * P, :])
            nc.scalar.dma_start(xq0[k][:], xT[k * P : (k + 1) * P, 0:Q])
        for k in range(KT):
            nc.scalar.dma_start(xq1[k][:], xT[k * P : (k + 1) * P, Q : 2 * Q])
        # Quarters 2-3 ride SWDGE kicks from the otherwise-idle gpsimd: a
        # third dispatcher, and a ring separate from q1 (w + out stores) and
        # q10 (x quarters 0-1), so the late x quarters stop arriving after
        # the PE needs them (the ~1 us PE stalls at the group transitions).
        xq = {0: None, 1: None}
        for q in range(2, 4):
            t_ = xpool.tile([P, KT * Q], BF16, tag=f"xq{q}")
            nc.gpsimd.dma_start(
                t_[:].rearrange("p (k b) -> p k b", k=KT),
                xT[:, q * Q : (q + 1) * Q].rearrange("(k p) b -> p k b", p=P),
            )
            xq[q] = t_

        def w_slice(k, half):
            if k == 0:
                return (w0lo if half == 0 else w0hi)[:]
            return wt[k - 1][:, half * 512 : (half + 1) * 512]

        def x_slice(k, bt):
            q, boff = divmod(bt * P, Q)
            if q == 0:
                return xq0[k][:, boff : boff + P]
            if q == 1:
                return xq1[k][:, boff : boff + P]
            return xq[q][:, k * Q + boff : k * Q + boff + P]

        # ACT Sqrt table preload, off the first epilogue's critical path.
        scrap = npool.tile([P, 1], F32, tag="scrap")
        nc.scalar.activation(scrap[:], warm[:, 0:1], SQRT)

        # ---- PE clock ramp: dummies bridge preamble-end -> first data ----
        pss = [psum.tile([P, C], F32, tag="ps", name=f"ps{i}") for i in range(4)]
        # Dummies start ~7.5 us (right after the gpsimd memset) and should
        # END just before the first (w0lo, xq0[0]) pair completes (~9.5 us):
        # undershooting idles the warm PE briefly; overshooting queues real
        # matmuls behind dummies. Cold N=512 matmuls pace at ~430-510 ns.
        for _ in range(5):
            nc.tensor.matmul(
                pss[3][:, 512:1024],
                warm[:, 0:P],
                warm[:],
                start=True,
                stop=True,
                skip_group_check=True,
            )

        def mm(bt, ps, k):
            lhs = x_slice(k, bt)
            nc.tensor.matmul(
                ps[:, 0:512], lhs, w_slice(k, 0), start=(k == 0), stop=(k == KT - 1)
            )
            nc.tensor.matmul(
                ps[:, 512:1024], lhs, w_slice(k, 1),
                start=(k == 0), stop=(k == KT - 1),
            )

        SQUARE = mybir.ActivationFunctionType.Square

        def epilogue(bt, ps):
            # Steady state (t0-t12): psum->bf16 logits copy alternating
            # ACT/DVE, one fused DVE square+row-accumulate from the bf16
            # logits, [P,1] alpha^2 add, ACT Sqrt, one fused logits||dist
            # store. Everything stays ~1-2.3us/engine per 2.56us tile.
            lg = lgpool.tile([P, 2 * C], BF16, tag="lgdt")
            sq = spool.tile([P, C], BF16, tag="sq")
            sn = npool.tile([P, 1], F32, tag="sn")
            snb = npool.tile([P, 1], F32, tag="snb")
            if bt % 2 == 0:
                nc.scalar.copy(lg[:, 0:1024], ps[:])
            else:
                nc.vector.tensor_copy(lg[:, 0:1024], ps[:])
            nc.vector.scalar_tensor_tensor(
                sq[:], lg[:, 0:1024], 1.0, lg[:, 0:1024],
                mult, mult, accum_out=sn[:],
            )
            nc.vector.tensor_scalar_add(snb[:], sn[:], ALPHA * ALPHA)
            nc.scalar.activation(
                lg[:, 1024:2048], lg[:, 0:1024], SQRT,
                bias=snb[:], scale=-2.0 * ALPHA,
            )
            nc.sync.dma_start(out[bt * P : (bt + 1) * P, :], lg[:])

        def drain_epilogue(bt, ps):
            # Drain tiles (t13-t14): everything dist-critical reads PSUM
            # directly -- ACT Square-with-accumulate then Sqrt-from-psum --
            # so no copy sits on the critical chain; the bf16 logits cast
            # rides the DVE into its OWN tile (shared tiles would add a
            # false tile-granular WAW dep) and ships separately.
            lgt = lgpool.tile([P, C], BF16, tag="lgt")
            dt_ = lgpool.tile([P, C], BF16, tag="dt")
            sq = spool.tile([P, C], BF16, tag="sq")
            snb = npool.tile([P, 1], F32, tag="snb")
            rows = slice(bt * P, (bt + 1) * P)
            sn = npool.tile([P, 1], F32, tag="sn")
            nc.scalar.activation(sq[:], ps[:], SQUARE, accum_out=sn[:])
            nc.vector.tensor_scalar_add(snb[:], sn[:], ALPHA * ALPHA)
            nc.vector.tensor_copy(lgt[:], ps[:])
            nc.sync.dma_start(out[rows, 0:1024], lgt[:])
            nc.scalar.activation(
                dt_[:], ps[:], SQRT, bias=snb[:], scale=-2.0 * ALPHA
            )
            nc.sync.dma_start(out[rows, 1024:2048], dt_[:])

        def final_tile(bt, ps):
            # Last tile runs a COLUMN-SPLIT k-loop: all 6 k-chunks for
            # columns 0-511 first, then for 512-1023. Half 0's Square+acc
            # (ACT), bf16 logits copy (DVE) and logits store overlap half 1's
            # matmuls, so after the very last matmul only half 1's epilogue
            # remains: Square+acc, combine the sn halves, and the two dist
            # Sqrts with stores kicked from two different queues.
            lgt = lgpool.tile([P, C], BF16, tag="lgt")
            dt_ = lgpool.tile([P, C], BF16, tag="dt")
            sq = spool.tile([P, C], BF16, tag="sq")
            sna = npool.tile([P, 1], F32, tag="sna")
            snb_ = npool.tile([P, 1], F32, tag="snb2")
            snt = npool.tile([P, 1], F32, tag="snt")
            rows = slice(bt * P, (bt + 1) * P)
            for k in range(KT):
                nc.tensor.matmul(
                    ps[:, 0:512], x_slice(k, bt), w_slice(k, 0),
                    start=(k == 0), stop=(k == KT - 1),
                )
            nc.scalar.activation(sq[:, 0:512], ps[:, 0:512], SQUARE, accum_out=sna[:])
            nc.vector.tensor_copy(lgt[:, 0:512], ps[:, 0:512])
            nc.sync.dma_start(out[rows, 0:512], lgt[:, 0:512])
            for k in range(KT):
                nc.tensor.matmul(
                    ps[:, 512:1024], x_slice(k, bt), w_slice(k, 1),
                    start=(k == 0), stop=(k == KT - 1),
                )
            nc.scalar.activation(
                sq[:, 512:1024], ps[:, 512:1024], SQUARE, accum_out=snb_[:]
            )
            nc.vector.tensor_copy(lgt[:, 512:1024], ps[:, 512:1024])
            nc.sync.dma_start(out[rows, 512:1024], lgt[:, 512:1024])
            nc.vector.scalar_tensor_tensor(
                snt[:], sna[:], ALPHA * ALPHA, snb_[:], add, add
            )
            nc.scalar.activation(
                dt_[:, 0:512], ps[:, 0:512], SQRT, bias=snt[:], scale=-2.0 * ALPHA
            )
            nc.sync.dma_start(out[rows, 1024:1536], dt_[:, 0:512])
            nc.scalar.activation(
                dt_[:, 512:1024], ps[:, 512:1024], SQRT,
                bias=snt[:], scale=-2.0 * ALPHA,
            )
            # kick from the ACT queue itself (ACT is an HWDGE engine) --
            # sync may still be draining the logits/half-1 kicks.
            nc.scalar.dma_start(out[rows, 1536:2048], dt_[:, 512:1024])

        # b-tiles 0-3 run k-major so each arriving (w_k, x_k q0) pair unlocks
        # 8 matmuls; later tiles run tile-major once DMA is ahead of the PE.
        for k in range(KT):
            for t in range(4):
                mm(t, pss[t], k)
        for t in range(4):
            epilogue(t, pss[t])

        # tiles 4-7 also run k-major (on the per-k q1 pieces) so the PE can
        # start as soon as each chunk lands instead of waiting for all of q1.
        pss2 = [psum.tile([P, C], F32, tag="ps", name=f"psB{i}") for i in range(4)]
        for k in range(KT):
            for i in range(4):
                mm(4 + i, pss2[i], k)
        for i in range(4):
            epilogue(4 + i, pss2[i])

        for bt in range(8, NBT):
            ps = psum.tile([P, C], F32, tag="ps")
            if bt == NBT - 1:
                final_tile(bt, ps)
                continue
            for k in range(KT):
                mm(bt, ps, k)
            if bt >= 13:
                drain_epilogue(bt, ps)
            else:
                epilogue(bt, ps)

    nc.compile()
    return nc


_NC = {}


def kernel(x, W, trace=False, _result_box=None):
    if "nc" not in _NC:
        _NC["nc"] = build()
    nc = _NC["nc"]

    x = np.ascontiguousarray(np.asarray(x, dtype=np.float32))
    W = np.ascontiguousarray(np.asarray(W, dtype=np.float32))
    prep = lambda a: np.asarray(a, dtype=ml_dtypes.bfloat16)
    wT = prep(np.ascontiguousarray(W.T))
    in_maps = [
        {
            "xT": prep(np.ascontiguousarray(x[i * BS : (i + 1) * BS, :].T)),
            "wT": wT,
        }
        for i in range(N_CORES)
    ]

    # The first execution of a freshly loaded NEFF has been seen to flake
    # (transient NRT_EXEC_UNIT_UNRECOVERABLE / corrupt output on this
    # fabric); do a throwaway warm-up exec with one retry, then the real run.
    try:
        run_bass_kernel_spmd(nc, in_maps, list(range(N_CORES)))
    except Exception:
        try:
            run_bass_kernel_spmd(nc, in_maps, list(range(N_CORES)))
        except Exception:
            pass

    res = run_bass_kernel_spmd(nc, in_maps, list(range(N_CORES)), trace=trace)
    if _result_box is not None:
        _result_box.append(res)

    outs = [np.asarray(res.results[i]["out"], dtype=np.float32) for i in range(N_CORES)]
    logits = np.concatenate([o[:, :C] for o in outs], axis=0)
    dist = np.concatenate([o[:, C:] for o in outs], axis=0)
    return logits, dist



# revision 14
# speedup vs baseline: 1.1598x; 1.0594x over previous
"""Trainium2 Bass kernel for nn_CACProjector (logits = x @ W^T, CAC distances).

Data-parallel over batch B across 8 NeuronCores; each core handles a
(768, 2048) column-slice xT of x^T plus a replicated W^T (768, 1024), all
shipped bf16:

  logits[b, c] = sum_d xT[d, b] * wT[d, c]      (PE, fp32 accumulate in PSUM)
  sq_norm[b]   = alpha^2 + sum_c logits[b, c]^2 (DVE tensor_tensor_reduce,
                                                 ONE pass: square + row-reduce
                                                 + alpha^2 init)
  dist[b, c]   = sqrt(sq_norm[b] - 2*alpha*logits[b, c])   (ACT Sqrt)

Engine split per 128-row b-tile: PE 12 matmuls (~2.6 us, pacing); the
PSUM->bf16 logits copy alternates between ACT and DVE (neither in-order queue
saturates); DVE runs one fused square+row-accumulate (scalar_tensor_tensor
accum_out) plus a [P,1] alpha^2 add; ACT runs the Sqrt. Tiles 13-15 are
"drain" tiles: Square-with-accumulate and Sqrt read PSUM directly on ACT so no
copy sits on the final dependency chain, and the last tile's Sqrt/store is
split in halves. (GPSIMD cannot access PSUM on TRN2, and tensor_tensor_reduce
dies at execution on this stack -- both found the hard way.)

Logits and dist are written into ONE [128, 2048] SBUF tile (lg || dist) and
shipped with a single DMA per tile into a fused [2048, 2048] DRAM tensor
(4 KB contiguous per row); the host splits the halves.

Input DMAs are issued in PE consumption order: (w_k, x_k quarter-0) pairs feed
a k-major opening over b-tiles 0-3 (each arriving pair unlocks 8 matmuls), and
x quarters 1-3 follow as single rearranged transfers. Dummy matmuls on a
zeroed tile bridge the NEFF preamble so the PE clock is ramped when real data
lands; a throwaway Sqrt preloads the ACT table off the first epilogue's path.

The final b-tile's epilogue is split into 512-column halves chained through
tensor_tensor_reduce's initial-value operand so the drain after the last
matmul is ~3 us instead of ~6; its dist store is kicked from the ACT queue
(ACT is an HWDGE engine) right behind the Sqrt.

d2 = ||l||^2 - 2a*l_j + a^2 >= (l_j - a)^2 >= 0 and with this data d2 ~ 1100,
so the reference's maximum(d2, 0) clamp is a no-op.
"""

import sys

sys.path.insert(0, "/opt/trn_rl_repo")

from contextlib import ExitStack

import ml_dtypes
import numpy as np

import concourse.tile as tile
from concourse import bacc, mybir
from concourse.bass_utils import run_bass_kernel_spmd

N_CORES = 8
B, D, C = 16384, 768, 1024
BS = B // N_CORES          # 2048 rows of B per core
P = 128                    # partition dim
KT = D // P                # 6 contraction chunks
NBT = BS // P              # 16 output row-tiles per core
Q = 512                    # b-quarter width (4 tiles)
ALPHA = 10.0

F32 = mybir.dt.float32
BF16 = mybir.dt.bfloat16


def build():
    nc = bacc.Bacc("TRN2", target_bir_lowering=False, debug=False)
    xT = nc.dram_tensor("xT", [D, BS], BF16, kind="ExternalInput").ap()
    wT = nc.dram_tensor("wT", [D, C], BF16, kind="ExternalInput").ap()
    out = nc.dram_tensor("out", [BS, 2 * C], BF16, kind="ExternalOutput").ap()

    mult, add = mybir.AluOpType.mult, mybir.AluOpType.add
    SQRT = mybir.ActivationFunctionType.Sqrt

    with tile.TileContext(nc) as tc, ExitStack() as ctx:
        xpool = ctx.enter_context(tc.tile_pool(name="xT", bufs=1))
        wpool = ctx.enter_context(tc.tile_pool(name="wT", bufs=1))
        psum = ctx.enter_context(tc.tile_pool(name="psum", bufs=4, space="PSUM"))
        lgpool = ctx.enter_context(tc.tile_pool(name="lgdt", bufs=8))
        spool = ctx.enter_context(tc.tile_pool(name="sq", bufs=4))
        npool = ctx.enter_context(tc.tile_pool(name="norms", bufs=10))

        # PE warm-up operand: memset on GPSIMD, whose framework preamble ends
        # earliest, FIRST in the gpsimd queue so the dummy matmuls can start
        # ~7.5 us. (Tile refuses reads from never-written tiles, so the
        # memset cannot be skipped.)
        warm = xpool.tile([P, Q], BF16, tag="warm")
        nc.gpsimd.memset(warm[:], 0)

        # ---- input DMAs, issued in PE consumption order ----
        # w kicks ride the sync queue, x kicks the scalar queue (also HWDGE):
        # two dispatchers halve the serial ~0.65us/kick cost at the front.
        # w0 is two separate TILES: tile-granular dep tracking would otherwise
        # gate the first lo-half matmul on the hi half's arrival.
        w0lo = wpool.tile([P, 512], BF16, tag="w0lo")
        w0hi = wpool.tile([P, 512], BF16, tag="w0hi")
        wt = [
            wpool.tile([P, C], BF16, tag=f"w{k}", name=f"w{k}")
            for k in range(1, KT)
        ]
        xq0 = [
            xpool.tile([P, Q], BF16, tag=f"xq0_{k}", name=f"xq0_{k}")
            for k in range(KT)
        ]
        # Quarter 1 ships as per-k pieces so the k-major group over tiles
        # 4-7 can start on partial data; quarters 2-3 land far ahead of use
        # and ride single rearranged transfers. xq1 kicks interleave behind
        # the tail of the q0 pieces so group B's early rounds aren't starved.
        xq1 = [
            xpool.tile([P, Q], BF16, tag=f"xq1_{k}", name=f"xq1_{k}")
            for k in range(KT)
        ]
        nc.sync.dma_start(w0lo[:], wT[0:P, 0:512])
        nc.scalar.dma_start(xq0[0][:], xT[0:P, 0:Q])
        nc.sync.dma_start(w0hi[:], wT[0:P, 512:1024])
        for k in range(1, KT):
            nc.sync.dma_start(wt[k - 1][:], wT[k * P : (k + 1) * P, :])
            nc.scalar.dma_start(xq0[k][:], xT[k * P : (k + 1) * P, 0:Q])
        for k in range(KT):
            nc.scalar.dma_start(xq1[k][:], xT[k * P : (k + 1) * P, Q : 2 * Q])
        # Quarters 2-3 ride the scalar HWDGE ring BEHIND xq0/xq1 (one logical
        # queue drains in order), so they cannot steal early SDMA arbitration
        # share from the w queue. (A SWDGE/gpsimd variant was tried: its ring
        # competed with w delivery during the critical first 12 us and the PE
        # start slipped ~2.5 us.)
        xq = {0: None, 1: None}
        for q in range(2, 4):
            t_ = xpool.tile([P, KT * Q], BF16, tag=f"xq{q}")
            nc.scalar.dma_start(
                t_[:].rearrange("p (k b) -> p k b", k=KT),
                xT[:, q * Q : (q + 1) * Q].rearrange("(k p) b -> p k b", p=P),
            )
            xq[q] = t_

        def w_slice(k, half):
            if k == 0:
                return (w0lo if half == 0 else w0hi)[:]
            return wt[k - 1][:, half * 512 : (half + 1) * 512]

        def x_slice(k, bt):
            q, boff = divmod(bt * P, Q)
            if q == 0:
                return xq0[k][:, boff : boff + P]
            if q == 1:
                return xq1[k][:, boff : boff + P]
            return xq[q][:, k * Q + boff : k * Q + boff + P]

        # ACT Sqrt table preload, off the first epilogue's critical path.
        scrap = npool.tile([P, 1], F32, tag="scrap")
        nc.scalar.activation(scrap[:], warm[:, 0:1], SQRT)

        # ---- PE clock ramp: dummies bridge preamble-end -> first data ----
        pss = [psum.tile([P, C], F32, tag="ps", name=f"ps{i}") for i in range(4)]
        # Dummies start ~7.5 us (right after the gpsimd memset) and should
        # END just before the first (w0lo, xq0[0]) pair COMPLETES. Each DMA
        # kick pays ~2 us of completion-receipt latency on top of transfer,
        # so the first pair's semaphores fire ~12.3 us -- 9 dummies at the
        # cold ~510 ns pace end ~12.1. Undershooting idles the PE (and lets
        # HAM re-throttle); overshooting queues real matmuls behind dummies.
        for _ in range(9):
            nc.tensor.matmul(
                pss[3][:, 512:1024],
                warm[:, 0:P],
                warm[:],
                start=True,
                stop=True,
                skip_group_check=True,
            )

        def mm(bt, ps, k):
            lhs = x_slice(k, bt)
            nc.tensor.matmul(
                ps[:, 0:512], lhs, w_slice(k, 0), start=(k == 0), stop=(k == KT - 1)
            )
            nc.tensor.matmul(
                ps[:, 512:1024], lhs, w_slice(k, 1),
                start=(k == 0), stop=(k == KT - 1),
            )

        SQUARE = mybir.ActivationFunctionType.Square

        def epilogue(bt, ps):
            # Steady state (t0-t12): psum->bf16 logits copy alternating
            # ACT/DVE, one fused DVE square+row-accumulate from the bf16
            # logits, [P,1] alpha^2 add, ACT Sqrt, one fused logits||dist
            # store. Everything stays ~1-2.3us/engine per 2.56us tile.
            lg = lgpool.tile([P, 2 * C], BF16, tag="lgdt")
            sq = spool.tile([P, C], BF16, tag="sq")
            sn = npool.tile([P, 1], F32, tag="sn")
            snb = npool.tile([P, 1], F32, tag="snb")
            if bt % 2 == 0:
                nc.scalar.copy(lg[:, 0:1024], ps[:])
            else:
                nc.vector.tensor_copy(lg[:, 0:1024], ps[:])
            nc.vector.scalar_tensor_tensor(
                sq[:], lg[:, 0:1024], 1.0, lg[:, 0:1024],
                mult, mult, accum_out=sn[:],
            )
            nc.vector.tensor_scalar_add(snb[:], sn[:], ALPHA * ALPHA)
            nc.scalar.activation(
                lg[:, 1024:2048], lg[:, 0:1024], SQRT,
                bias=snb[:], scale=-2.0 * ALPHA,
            )
            nc.sync.dma_start(out[bt * P : (bt + 1) * P, :], lg[:])

        def drain_epilogue(bt, ps):
            # Drain tiles (t13-t14): everything dist-critical reads PSUM
            # directly -- ACT Square-with-accumulate then Sqrt-from-psum --
            # so no copy sits on the critical chain; the bf16 logits cast
            # rides the DVE into its OWN tile (shared tiles would add a
            # false tile-granular WAW dep) and ships separately.
            lgt = lgpool.tile([P, C], BF16, tag="lgt")
            dt_ = lgpool.tile([P, C], BF16, tag="dt")
            sq = spool.tile([P, C], BF16, tag="sq")
            snb = npool.tile([P, 1], F32, tag="snb")
            rows = slice(bt * P, (bt + 1) * P)
            sn = npool.tile([P, 1], F32, tag="sn")
            nc.scalar.activation(sq[:], ps[:], SQUARE, accum_out=sn[:])
            nc.vector.tensor_scalar_add(snb[:], sn[:], ALPHA * ALPHA)
            nc.vector.tensor_copy(lgt[:], ps[:])
            nc.sync.dma_start(out[rows, 0:1024], lgt[:])
            nc.scalar.activation(
                dt_[:], ps[:], SQRT, bias=snb[:], scale=-2.0 * ALPHA
            )
            nc.sync.dma_start(out[rows, 1024:2048], dt_[:])

        def final_tile(bt, ps):
            # Last tile runs a COLUMN-SPLIT k-loop: all 6 k-chunks for
            # columns 0-511 first, then for 512-1023. Half 0's Square+acc
            # (ACT), bf16 logits copy (DVE) and logits store overlap half 1's
            # matmuls, so after the very last matmul only half 1's epilogue
            # remains: Square+acc, combine the sn halves, and the two dist
            # Sqrts with stores kicked from two different queues.
            lgt = lgpool.tile([P, C], BF16, tag="lgt")
            dt_ = lgpool.tile([P, C], BF16, tag="dt")
            sq = spool.tile([P, C], BF16, tag="sq")
            sna = npool.tile([P, 1], F32, tag="sna")
            snb_ = npool.tile([P, 1], F32, tag="snb2")
            snt = npool.tile([P, 1], F32, tag="snt")
            rows = slice(bt * P, (bt + 1) * P)
            for k in range(KT):
                nc.tensor.matmul(
                    ps[:, 0:512], x_slice(k, bt), w_slice(k, 0),
                    start=(k == 0), stop=(k == KT - 1),
                )
            nc.scalar.activation(sq[:, 0:512], ps[:, 0:512], SQUARE, accum_out=sna[:])
            nc.vector.tensor_copy(lgt[:, 0:512], ps[:, 0:512])
            nc.sync.dma_start(out[rows, 0:512], lgt[:, 0:512])
            for k in range(KT):
                nc.tensor.matmul(
                    ps[:, 512:1024], x_slice(k, bt), w_slice(k, 1),
                    start=(k == 0), stop=(k == KT - 1),
                )
            nc.scalar.activation(
                sq[:, 512:1024], ps[:, 512:1024], SQUARE, accum_out=snb_[:]
            )
            # sn-combine FIRST on the strict-FIFO DVE queue: the half-1
            # logits cast (not dist-critical) must not delay it.
            nc.vector.scalar_tensor_tensor(
                snt[:], sna[:], ALPHA * ALPHA, snb_[:], add, add
            )
            nc.vector.tensor_copy(lgt[:, 512:1024], ps[:, 512:1024])
            nc.sync.dma_start(out[rows, 512:1024], lgt[:, 512:1024])
            nc.scalar.activation(
                dt_[:, 0:512], ps[:, 0:512], SQRT, bias=snt[:], scale=-2.0 * ALPHA
            )
            nc.sync.dma_start(out[rows, 1024:1536], dt_[:, 0:512])
            nc.scalar.activation(
                dt_[:, 512:1024], ps[:, 512:1024], SQRT,
                bias=snt[:], scale=-2.0 * ALPHA,
            )
            # kick from the ACT queue itself (ACT is an HWDGE engine) --
            # sync may still be draining the logits/half-1 kicks.
            nc.scalar.dma_start(out[rows, 1536:2048], dt_[:, 512:1024])

        # b-tiles 0-3 run k-major so each arriving (w_k, x_k q0) pair unlocks
        # 8 matmuls; later tiles run tile-major once DMA is ahead of the PE.
        for k in range(KT):
            for t in range(4):
                mm(t, pss[t], k)
        for t in range(4):
            epilogue(t, pss[t])

        # tiles 4-7 also run k-major (on the per-k q1 pieces) so the PE can
        # start as soon as each chunk lands instead of waiting for all of q1.
        pss2 = [psum.tile([P, C], F32, tag="ps", name=f"psB{i}") for i in range(4)]
        for k in range(KT):
            for i in range(4):
                mm(4 + i, pss2[i], k)
        for i in range(4):
            epilogue(4 + i, pss2[i])

        for bt in range(8, NBT):
            ps = psum.tile([P, C], F32, tag="ps")
            if bt == NBT - 1:
                final_tile(bt, ps)
                continue
            for k in range(KT):
                mm(bt, ps, k)
            if bt >= 13:
                drain_epilogue(bt, ps)
            else:
                epilogue(bt, ps)

    nc.compile()
    return nc


_NC = {}


def kernel(x, W, trace=False, _result_box=None):
    if "nc" not in _NC:
        _NC["nc"] = build()
    nc = _NC["nc"]

    x = np.ascontiguousarray(np.asarray(x, dtype=np.float32))
    W = np.ascontiguousarray(np.asarray(W, dtype=np.float32))
    prep = lambda a: np.asarray(a, dtype=ml_dtypes.bfloat16)
    wT = prep(np.ascontiguousarray(W.T))
    in_maps = [
        {
            "xT": prep(np.ascontiguousarray(x[i * BS : (i + 1) * BS, :].T)),
            "wT": wT,
        }
        for i in range(N_CORES)
    ]

    # The first execution of a freshly loaded NEFF has been seen to flake
    # (transient NRT_EXEC_UNIT_UNRECOVERABLE / corrupt output on this
    # fabric); do a throwaway warm-up exec with one retry, then the real run.
    try:
        run_bass_kernel_spmd(nc, in_maps, list(range(N_CORES)))
    except Exception:
        try:
            run_bass_kernel_spmd(nc, in_maps, list(range(N_CORES)))
        except Exception:
            pass

    res = run_bass_kernel_spmd(nc, in_maps, list(range(N_CORES)), trace=trace)
    if _result_box is not None:
        _result_box.append(res)

    outs = [np.asarray(res.results[i]["out"], dtype=np.float32) for i in range(N_CORES)]
    logits = np.concatenate([o[:, :C] for o in outs], axis=0)
    dist = np.concatenate([o[:, C:] for o in outs], axis=0)
    return logits, dist

